# revision 1
# baseline (speedup 1.0000x reference)
"""GQA causal attention block (q/k/v proj + softmax attention + out proj),
tensor-parallel over 8 NeuronCores.

Reference semantics (fp32):
  q = x @ Wq  -> [s, 32, 64];  k,v = x @ Wk/Wv -> [s, 8, 64]
  GQA repeat kv x4, causal softmax(q k^T / 8) @ v, concat -> @ Wo + bo

Sharding: core i owns q-heads 4i..4i+3 and kv-head i (kv groups aligned),
Wo rows 256i..256i+256.  Each core computes a full-shape partial output;
host sums the 8 partials and adds bo.

Per-core layout choices (everything keeps the contraction dim on SBUF
partitions so no transposes are needed anywhere except v):
  xT [2048, 4096]   host-pretransposed activations (replicated)
  qT2 pairs [128, s]: rows 0:64 head 2p, 64:128 head 2p+1
  kvT [128, s]: rows 0:64 = vT, 64:128 = kT (single proj series)
  kTp [128, s]: kT duplicated on both halves (for 2-head row-tiled scores)
  scoresT [sk, sq] blocks so the exp'd tile feeds the ctx matmul as the
  moving operand; v kept natural [sk, 64+1] with a ones column so the
  softmax denominator falls out of the ctx matmul as row 64.
"""

import numpy as np

HEAD_DIM = 64
N_CORES = 8
S = 4096
D_IN = 2048
D_OUT = 2048
BLK = 128  # sk block (partition dim of scoresT tiles)
CHUNK = 512  # sq chunk width
SCALE = 1.0 / 8.0  # 1/sqrt(64)

_NC_CACHE = {}


def _build(s):
    from contextlib import ExitStack

    import concourse.mybir as mybir
    import concourse.tile as tile
    from concourse import bacc
    from concourse.bass import ts
    from concourse.masks import make_identity

    f32 = mybir.dt.float32
    f16 = mybir.dt.float16
    Exp = mybir.ActivationFunctionType.Exp
    nchunk = s // CHUNK
    nblk = s // BLK

    nc = bacc.Bacc("TRN2", target_bir_lowering=False, debug=False)
    xT = nc.dram_tensor("xT", [D_IN, s], f16, kind="ExternalInput")
    wq = nc.dram_tensor("wq", [2, D_IN, 128], f16, kind="ExternalInput")
    wkv = nc.dram_tensor("wkv", [D_IN, 128], f16, kind="ExternalInput")
    wo = nc.dram_tensor("wo", [2, 128, D_OUT], f16, kind="ExternalInput")
    out = nc.dram_tensor("out", [s, D_OUT], f16, kind="ExternalOutput")

    with tile.TileContext(nc) as tc, ExitStack() as ctx:
        singles = ctx.enter_context(tc.tile_pool(name="singles", bufs=1))
        wq_sb = [singles.tile([128, 16, 128], f16, tag=f"wq{p}", name=f"wq_sb{p}") for p in range(2)]
        wkv_sb = singles.tile([128, 16, 128], f16, tag="wkv", name="wkv_sb")
        wo_sb = [singles.tile([128, D_OUT], f16, tag=f"wo{p}", name=f"wo_sb{p}") for p in range(2)]
        kvT = singles.tile([128, s], f16, tag="kvT", name="kvT")
        kTp = singles.tile([128, s], f16, tag="kTp", name="kTp")
        v_sb = singles.tile([128, nblk, HEAD_DIM + 1], f16, tag="v", name="v_sb")
        ident = singles.tile([128, 128], f16, tag="ident", name="ident")
        ones64 = singles.tile([128, HEAD_DIM], f16, tag="ones", name="ones64")

        for p in range(2):
            nc.sync.dma_start(
                out=wq_sb[p], in_=wq[p, :, :].rearrange("(kc p) m -> p kc m", p=128)
            )
            nc.sync.dma_start(out=wo_sb[p], in_=wo[p, :, :])
        nc.sync.dma_start(
            out=wkv_sb, in_=wkv.rearrange("(kc p) m -> p kc m", p=128)
        )
        make_identity(nc, ident)
        nc.vector.memset(ones64, 0.0)
        nc.vector.memset(ones64[64:65, :], 1.0 / 64.0)
        nc.vector.memset(v_sb[:, :, HEAD_DIM : HEAD_DIM + 1], 1.0 / 64.0)

        xt_pool = ctx.enter_context(tc.tile_pool(name="xt", bufs=1))
        qt_pool = ctx.enter_context(tc.tile_pool(name="qt", bufs=2))
        exp_pool = ctx.enter_context(tc.tile_pool(name="exp", bufs=2))
        ctxq_pool = ctx.enter_context(tc.tile_pool(name="ctxq", bufs=2))
        outsb_pool = ctx.enter_context(tc.tile_pool(name="outsb", bufs=2))
        small_pool = ctx.enter_context(tc.tile_pool(name="small", bufs=2))
        ps_scores = ctx.enter_context(tc.tile_pool(name="pssc", bufs=2, space="PSUM"))
        ps_ctx = ctx.enter_context(tc.tile_pool(name="psctx", bufs=2, space="PSUM"))
        ps_misc = ctx.enter_context(tc.tile_pool(name="psmisc", bufs=2, space="PSUM"))

        for c in range(nchunk):
            cs = ts(c, CHUNK)
            # ---- q/k/v projections for this sq chunk ----
            xt = xt_pool.tile([128, 16, CHUNK], f16, tag="xt", name="xt")
            nc.sync.dma_start(
                out=xt, in_=xT[:, cs].rearrange("(kc p) n -> p kc n", p=128)
            )
            qt = []
            for p in range(2):
                pq = ps_misc.tile([128, CHUNK], f32, tag="misc", name="psmisc_t")
                for k in range(16):
                    nc.tensor.matmul(
                        pq, wq_sb[p][:, k, :], xt[:, k, :],
                        start=(k == 0), stop=(k == 15),
                    )
                qtp = qt_pool.tile([128, CHUNK], f16, tag=f"qt{p}", name=f"qt{p}")
                nc.vector.tensor_copy(out=qtp, in_=pq)
                qt.append(qtp)
            pkv = ps_misc.tile([128, CHUNK], f32, tag="misc", name="psmisc_t")
            for k in range(16):
                nc.tensor.matmul(
                    pkv, wkv_sb[:, k, :], xt[:, k, :],
                    start=(k == 0), stop=(k == 15),
                )
            nc.vector.tensor_copy(out=kvT[:, cs], in_=pkv)
            # duplicate kT onto both partition halves (row-tiled scores)
            nc.sync.dma_start(out=kTp[0:64, cs], in_=kvT[64:128, cs])
            nc.sync.dma_start(out=kTp[64:128, cs], in_=kvT[64:128, cs])
            # v natural [sk, 64] via PE transpose of vT blocks
            for j in range(4):
                blk = 4 * c + j
                pv = ps_misc.tile([128, CHUNK], f16, tag="misc", name="psmisc_t")
                nc.tensor.transpose(
                    pv[:, 0:HEAD_DIM], kvT[0:64, ts(blk, BLK)], ident[0:64, 0:64]
                )
                nc.vector.tensor_copy(out=v_sb[:, blk, 0:HEAD_DIM], in_=pv[:, 0:HEAD_DIM])

            # ---- attention for this chunk, two head pairs ----
            ctxq = []
            for p in range(2):
                pctx = [ps_ctx.tile([128, CHUNK], f32, tag="ctx", name="pctx_t") for _ in range(2)]
                last_b = 4 * c + 3
                for g in range(2 * (c + 1)):
                    sc = [
                        ps_scores.tile([128, 2 * CHUNK], f32, tag="scores", name="sc_t")
                        for _ in range(2)
                    ]
                    for t in range(2):
                        b = 2 * g + t
                        nc.tensor.matmul(
                            sc[0][:, ts(t, CHUNK)],
                            kTp[0:64, ts(b, BLK)], qt[p][0:64, :],
                            start=True, stop=True,
                        )
                        nc.tensor.matmul(
                            sc[1][:, ts(t, CHUNK)],
                            kTp[64:128, ts(b, BLK)], qt[p][64:128, :],
                            start=True, stop=True,
                        )
                    for h in range(2):
                        ex = exp_pool.tile([128, 2 * CHUNK], f16, tag=f"exp{h}", name=f"ex{h}")
                        nc.scalar.activation(out=ex, in_=sc[h], func=Exp, scale=SCALE)
                        for t in range(2):
                            b = 2 * g + t
                            jd = b - 4 * c
                            if jd >= 0:
                                # causal mask inside diagonal blocks:
                                # keep where q - k - 128*jd >= 0
                                nc.gpsimd.affine_select(
                                    out=ex[:, ts(t, CHUNK)],
                                    in_=ex[:, ts(t, CHUNK)],
                                    compare_op=mybir.AluOpType.is_ge,
                                    fill=0.0,
                                    base=-BLK * jd,
                                    pattern=[[1, CHUNK]],
                                    channel_multiplier=-1,
                                )
                        for t in range(2):
                            b = 2 * g + t
                            nc.tensor.matmul(
                                pctx[h][: HEAD_DIM + 1, :],
                                v_sb[:, b, :], ex[:, ts(t, CHUNK)],
                                start=(b == 0), stop=(b == last_b),
                            )
                # normalize: row 64 of pctx is the softmax denominator
                # (scaled by 1/64); 1/64 * recip broadcast to 64 rows via a
                # K=1 matmul against the 1/64-valued ones row.
                ctxqp = ctxq_pool.tile([128, CHUNK], f16, tag=f"ctxq{p}", name=f"ctxq{p}")
                tmpb = small_pool.tile([128, CHUNK], f16, tag="tmpb", name="tmpb")
                for h in range(2):
                    recip = small_pool.tile([128, CHUNK], f32, tag="recip", name="recip")
                    recip16 = small_pool.tile([128, CHUNK], f16, tag="recip16", name="recip16")
                    nc.vector.reciprocal(
                        out=recip[64:65, :], in_=pctx[h][64:65, :]
                    )
                    nc.vector.tensor_copy(out=recip16[64:65, :], in_=recip[64:65, :])
                    psb = ps_misc.tile([128, CHUNK], f32, tag="misc", name="psmisc_t")
                    nc.tensor.matmul(
                        psb[0:64, :], ones64[64:65, :], recip16[64:65, :],
                        start=True, stop=True,
                    )
                    bsb = small_pool.tile([128, CHUNK], f32, tag="bsb", name="bsb")
                    nc.vector.tensor_copy(out=bsb[0:64, :], in_=psb[0:64, :])
                    dst = ctxqp[0:64, :] if h == 0 else tmpb[0:64, :]
                    nc.vector.tensor_mul(dst, pctx[h][0:64, :], bsb[0:64, :])
                # move head B rows into partitions 64:128 (cross-partition DMA)
                nc.sync.dma_start(out=ctxqp[64:128, :], in_=tmpb[0:64, :])
                ctxq.append(ctxqp)

            # ---- output projection for this chunk ----
            for jj in range(4):
                osb = outsb_pool.tile([128, D_OUT], f16, tag="osb", name="osb")
                for n in range(D_OUT // 512):
                    po = ps_misc.tile([128, CHUNK], f32, tag="misc", name="psmisc_t")
                    for p in range(2):
                        nc.tensor.matmul(
                            po, ctxq[p][:, ts(jj, 128)], wo_sb[p][:, ts(n, 512)],
                            start=(p == 0), stop=(p == 1),
                        )
                    nc.vector.tensor_copy(out=osb[:, ts(n, 512)], in_=po)
                nc.sync.dma_start(out=out[ts(4 * c + jj, 128), :], in_=osb)

    nc.compile()
    return nc


def _get_nc(s):
    if s not in _NC_CACHE:
        _NC_CACHE[s] = _build(s)
    return _NC_CACHE[s]


def _in_maps(x, Wq, Wk, Wv, Wo, s):
    xT = np.ascontiguousarray(np.asarray(x, np.float32)[0].T.astype(np.float16))
    Wq = np.asarray(Wq, np.float32).astype(np.float16)
    Wk = np.asarray(Wk, np.float32).astype(np.float16)
    Wv = np.asarray(Wv, np.float32).astype(np.float16)
    Wo = np.asarray(Wo, np.float32).astype(np.float16)
    maps = []
    for i in range(N_CORES):
        wq_i = np.ascontiguousarray(
            Wq[:, i * 256 : (i + 1) * 256].reshape(D_IN, 2, 128).transpose(1, 0, 2)
        )
        wkv_i = np.ascontiguousarray(
            np.concatenate(
                [Wv[:, i * 64 : (i + 1) * 64], Wk[:, i * 64 : (i + 1) * 64]], axis=1
            )
        )
        wo_i = np.ascontiguousarray(
            Wo[i * 256 : (i + 1) * 256, :].reshape(2, 128, D_OUT)
        )
        maps.append({"xT": xT, "wq": wq_i, "wkv": wkv_i, "wo": wo_i})
    return maps


def run(x, Wq, Wk, Wv, Wo, bo, s=S, **spmd_kwargs):
    """Builds (cached), runs on 8 cores, returns (full_output, BassKernelResults)."""
    from concourse.bass_utils import run_bass_kernel_spmd

    nc = _get_nc(s)
    maps = _in_maps(x, Wq, Wk, Wv, Wo, s)
    res = run_bass_kernel_spmd(nc, maps, core_ids=list(range(N_CORES)), **spmd_kwargs)
    acc = np.zeros((s, D_OUT), np.float64)
    for r in res.results:
        acc += r["out"].astype(np.float64)
    full = (acc + np.asarray(bo, np.float64)[None, :]).astype(np.float32)[None]
    return full, res


def kernel(x, Wq, Wk, Wv, Wo, bo):
    out, _ = run(x, Wq, Wk, Wv, Wo, bo)
    return out

